# revision 26
# baseline (speedup 1.0000x reference)
"""Trainium2 Bass kernel for nn_Loss_1_8323646620405 (multi-head BCE/CCE loss).

Data-parallel over batch: 8 cores x 8 batches. Host re-encodes inputs into a
single plane-major uint16 array per core (packbits labels + bf16 pred planes,
binary-head planes pre-shifted by -0.5) so each chunk is ONE DMA. Device:
  s    = any(y)        -> u = s-0.5            (DVE tensor_scalar on int16)
  L_s  = ln(0.5+2(ps-.5)u)                      (ACT Ln, accum -> acc1)
  ppe  = 0.5-2(pp-.5)(y0-.5), phe likewise      (DVE TT + ACT affine copy)
  Psel = P3[point idx], Qsel = Q4[serve idx]    (copy_predicated cascades, Pool)
  Pi   = ppe*phe*Psel*Qsel ; L_Pi = ln(Pi)      (DVE TT, ACT Ln)
  acc2 = sum s*(C2*L_s + L_Pi)                  (DVE STT accum)
loss = -(W0*acc1 + acc2)/(B*S) summed on host.
"""

import numpy as np
import ml_dtypes

import concourse.bass as bass
import concourse.mybir as mybir
import concourse.tile as tile
from concourse.bass_utils import run_bass_kernel_spmd

# ---- walrus single-wait workaround ----------------------------------------
# This container's walrus build encodes at most ONE semaphore wait per
# instruction ('Too many sync wait commands'). Tile's scheduler freely
# attaches N waits to one instruction. Two patches:
#  1. postorder_instruction_blocks wrapper: split any instruction carrying
#     >1 wait -- extra waits move to same-engine NoOps inserted before it.
#  2. _drain_and_barrier: one drain per outstanding logical processor.
import bass_rust
from concourse.tile_cfg import postorder_instruction_blocks as _orig_post

_DMA_PROC_START = 10  # Collectives/DMASW*/DMAHW* procs inc by 16 per tick
_nop_ctr = [0]


def _split_waits_in_list(insts):
    out = []
    for ins in insts:
        si = getattr(ins, "sync_info", None)
        waits = list(si.on_wait) if si is not None else []
        if len(waits) > 1:
            for w in waits[:-1]:
                _nop_ctr[0] += 1
                nop = mybir.InstNoOp(name=f"WSPL-{_nop_ctr[0]}", ins=[], outs=[])
                nop.engine = ins.engine
                nop.sync_info = bass_rust.SyncInfo(on_wait=[w], on_update=[])
                out.append(nop)
            ins.sync_info = bass_rust.SyncInfo(
                on_wait=[waits[-1]], on_update=list(si.on_update)
            )
        out.append(ins)
    return out


def _patched_post(instructions, start_bb, output):
    for k in list(instructions.keys()):
        instructions[k] = _split_waits_in_list(instructions[k])
    return _orig_post(instructions, start_bb, output)


def _split_drain_and_barrier(self, tick_clock, wait_clock):
    gc = tick_clock.global_clock
    alloc = wait_clock.sems.allocated()
    for proc in sorted(alloc):
        tick = gc.peek_next(proc) - 1
        if tick <= 0:
            continue
        scale = 16 if proc >= _DMA_PROC_START else 1
        d = self.nc.sync.drain()
        d.wait_op(alloc[proc], tick * scale, "sem-ge")

    self.nc.all_engine_barrier()
    popped = self.nc._tile_sem_poison_stack.pop()
    assert popped is self._sem_poison
    self.nc.clear_and_free_semaphores(list(self.sems.allocated().values()))
    self.nc.all_engine_barrier()


tile.postorder_instruction_blocks = _patched_post
tile.TileContext._drain_and_barrier = _split_drain_and_barrier

# ---- problem constants -----------------------------------------------------
B, S, F = 64, 32768, 9
W0, W1 = 0.51, 19.05
C2 = W1 - W0

NCORES = 8
B_LOC = B // NCORES          # 8 batches per core
N = B_LOC * S                # 262144 elements per core
P = 128                      # SBUF partitions
FD = N // P                  # 2048 free-dim elements per partition
NPLANES = 11                 # 0: labels, 1-10: bf16 pred planes
# >=256-col chunks keep DMA rows >=512B (full rate); tiny last chunk
# shortens the serial drain of the final dependency chain
CHUNKS = [(0, 256), (256, 640), (896, 640), (1536, 384), (1920, 128)]
K = len(CHUNKS)

f32 = mybir.dt.float32
bf16 = mybir.dt.bfloat16
i16 = mybir.dt.int16
u16 = mybir.dt.uint16
Alu = mybir.AluOpType
Act = mybir.ActivationFunctionType


def _build_nc() -> bass.Bass:
    nc = bass.Bass()

    # const AP for Ln bias=0.5 (same pattern as Bass.__init__ consts)
    c05 = nc.alloc_sbuf_tensor("const-float32-0.5", [P, 1], f32)
    nc.gpsimd.memset(c05.ap(), 0.5)
    nc.const_aps.aps[(f32, 0.5)] = c05.ap()
    nc.all_engine_barrier()

    D_d = nc.declare_dram_parameter("D", [NPLANES * N], bf16, isOutput=False)
    acc_d = nc.declare_dram_parameter("acc", [P, 4 * K], f32, isOutput=True)

    Dv = D_d.rearrange("(k p c) -> p k c", k=NPLANES, p=P)

    with tile.TileContext(nc) as tc:
        with (
            tc.tile_pool(name="io", bufs=2) as io,
            tc.tile_pool(name="tmp", bufs=2) as tp,
            tc.tile_pool(name="acc", bufs=1) as ac,
        ):
            accT = ac.tile([P, 4 * K], f32)
            acc1 = accT[:, 0:K]       # sum L_s            (ACT accum)
            acc2a = accT[:, K : 2 * K]    # sum u*L_s      (DVE accum)
            acc2b = accT[:, 2 * K : 3 * K]  # sum u*L_Pi   (DVE accum)
            acc3 = accT[:, 3 * K : 4 * K]   # sum L_Pi     (ACT accum)

            tiles = {}

            def load(k):
                off, sz = CHUNKS[k]
                T = io.tile([P, NPLANES, sz], bf16, tag="T")
                nc.sync.dma_start(T[:], Dv[:, :, off : off + sz])
                tiles[k] = T

            load(0)
            for k, (off, sz) in enumerate(CHUNKS):
                T = tiles.pop(k)
                if k + 1 < K:
                    load(k + 1)
                yb = T[:, 0, :].bitcast(u16)
                psh = T[:, 1, :]   # ps - 0.5 (host pre-shifted)
                pph = T[:, 2, :]   # pp - 0.5
                phh = T[:, 3, :]   # ph - 0.5
                P0 = T[:, 4, :]
                P1 = T[:, 5, :]
                Psel = T[:, 6, :]  # P2 plane, overwritten in place by selects
                Q0 = T[:, 7, :]
                Q1 = T[:, 8, :]
                Q2 = T[:, 9, :]
                Qsel = T[:, 10, :]  # Q3 plane, overwritten in place

                u = tp.tile([P, sz], bf16, tag="u")
                v0 = tp.tile([P, sz], bf16, tag="v0")
                u7m = tp.tile([P, sz], u16, tag="u7m")
                v7 = tp.tile([P, sz], bf16, tag="v7")
                y5m = tp.tile([P, sz], u16, tag="y5m")
                y4m = tp.tile([P, sz], u16, tag="y4m")
                y6m = tp.tile([P, sz], u16, tag="y6m")
                y3m = tp.tile([P, sz], u16, tag="y3m")
                y2m = tp.tile([P, sz], u16, tag="y2m")
                m_s = tp.tile([P, sz], bf16, tag="m_s")
                Xp = tp.tile([P, sz], bf16, tag="Xp")
                Xh = tp.tile([P, sz], bf16, tag="Xh")
                L_s = tp.tile([P, sz], bf16, tag="L_s")
                ppe = tp.tile([P, sz], bf16, tag="ppe")
                phe = tp.tile([P, sz], bf16, tag="phe")
                pr1 = tp.tile([P, sz], bf16, tag="pr1")
                pr2 = tp.tile([P, sz], bf16, tag="pr2")
                Pi = tp.tile([P, sz], bf16, tag="Pi")
                L_Pi = tp.tile([P, sz], bf16, tag="L_Pi")
                sLs = tp.tile([P, sz], bf16, tag="sLs")
                sR = tp.tile([P, sz], bf16, tag="sR")
                dq1 = tp.tile([P, sz], bf16, tag="dq1")
                dq2 = tp.tile([P, sz], bf16, tag="dq2")

                # label-derived masks. yb is uint16 with host bit layout:
                # bit15=y0, bit14=y7, bits0-4 = y2,y3,y4,y5,y6. Unsigned
                # compares extract the high bits in one tensor_scalar each
                # (walrus requires op0/op1 to share the bitwise/arith class).
                nc.vector.tensor_scalar(u[:], yb, 0, 0.5, Alu.is_gt, Alu.subtract)
                nc.vector.tensor_scalar(v0[:], yb, 32767, 0.5, Alu.is_gt, Alu.subtract)
                nc.vector.tensor_scalar(u7m[:], yb, 16384, None, Alu.bitwise_and)
                nc.vector.tensor_scalar(v7[:], u7m[:], 8192.0, None, Alu.subtract)
                nc.vector.tensor_scalar(y5m[:], yb, 8, None, Alu.bitwise_and)
                nc.vector.tensor_scalar(y4m[:], yb, 4, None, Alu.bitwise_and)
                nc.vector.tensor_scalar(y6m[:], yb, 16, None, Alu.bitwise_and)
                nc.vector.tensor_scalar(y3m[:], yb, 2, None, Alu.bitwise_and)
                nc.vector.tensor_scalar(y2m[:], yb, 1, None, Alu.bitwise_and)

                # stroke: L_s = ln(0.5 + 2*(ps-0.5)*u), accumulate per chunk
                nc.vector.tensor_tensor(m_s[:], psh, u[:], op=Alu.mult)
                nc.scalar.activation(L_s[:], m_s[:], Act.Ln, bias=0.5, scale=2.0,
                                     accum_out=acc1[:, k : k + 1])
                # player: v0 = y0-0.5, Xp = (pp-.5)(y0-.5), ppe = 0.5-2*Xp
                nc.vector.tensor_tensor(Xp[:], pph, v0[:], op=Alu.mult)
                nc.scalar.activation(ppe[:], Xp[:], Act.Copy, bias=0.5, scale=-2.0)
                # hand: v7 = 16384*(y7-0.5), phe = 0.5 - Xh/8192
                # (last chunk runs its Pool ops on DVE: the tail chain is tiny
                # there and cross-engine hops would serialize the drain)
                tt_eng = nc.vector if k == K - 1 else nc.gpsimd
                tt_eng.tensor_tensor(Xh[:], phh, v7[:], op=Alu.mult)
                nc.scalar.activation(phe[:], Xh[:], Act.Copy, bias=0.5, scale=-1.0 / 8192.0)
                # point select: y4 ? P0 : (y5 ? P1 : P2)  (in place on P2 plane;
                # copy_predicated is DVE-only on Trn2)
                nc.vector.copy_predicated(Psel, y5m[:], P1)
                nc.vector.copy_predicated(Psel, y4m[:], P0)
                # serve select: y2 ? Q0 : (y3 ? Q1 : (y6 ? Q2 : Q3))
                nc.vector.copy_predicated(Qsel, y6m[:], Q2)
                nc.vector.copy_predicated(Qsel, y3m[:], Q1)
                nc.vector.copy_predicated(Qsel, y2m[:], Q0)
                # Pi = ppe*phe*Psel*Qsel ; L_Pi = ln(Pi)
                tt_eng.tensor_tensor(pr1[:], ppe[:], phe[:], op=Alu.mult)
                nc.vector.tensor_tensor(pr2[:], Psel, Qsel, op=Alu.mult)
                tt_eng.tensor_tensor(Pi[:], pr1[:], pr2[:], op=Alu.mult)
                nc.scalar.activation(L_Pi[:], Pi[:], Act.Ln,
                                     accum_out=acc3[:, k : k + 1])
                # masked accums via u=s-0.5: sum s*X = sum u*X + sum X / 2
                # (host adds the acc1/acc3 halves). TT on Pool, reduce on ACT.
                tt_eng.tensor_tensor(sLs[:], u[:], L_s[:], op=Alu.mult)
                nc.scalar.activation(dq1[:], sLs[:], Act.Copy,
                                     accum_out=acc2a[:, k : k + 1])
                tt_eng.tensor_tensor(sR[:], u[:], L_Pi[:], op=Alu.mult)
                nc.scalar.activation(dq2[:], sR[:], Act.Copy,
                                     accum_out=acc2b[:, k : k + 1])

            nc.sync.dma_start(acc_d[:], accT[:])

    return nc


_NC_CACHE = None


def _get_nc():
    global _NC_CACHE
    if _NC_CACHE is None:
        _NC_CACHE = _build_nc()
    return _NC_CACHE


def _shard_inputs(inputs):
    bf = ml_dtypes.bfloat16
    # bf16 planes (computed once on full arrays, then sliced per core)
    planes_full = [
        (inputs["y_pred_stroke"][..., 0] - 0.5).astype(bf).view(np.uint16),
        (inputs["y_pred_player"][..., 0] - 0.5).astype(bf).view(np.uint16),
        (inputs["y_pred_hand"][..., 0] - 0.5).astype(bf).view(np.uint16),
    ] + [
        inputs["y_pred_point"][..., j].astype(bf).view(np.uint16) for j in range(3)
    ] + [
        inputs["y_pred_serve"][..., j].astype(bf).view(np.uint16) for j in range(4)
    ]
    # labels -> uint16 with layout: bits0-4 = y2,y3,y4,y5,y6; bit5=y1;
    # bit6=y8; bit14=y7; bit15=y0 (high bits enable unsigned-compare tricks)
    y = inputs["y_target"].astype(np.uint8)
    cols = np.zeros(y.shape[:-1] + (16,), np.uint8)
    for bit, j in [(0, 2), (1, 3), (2, 4), (3, 5), (4, 6), (5, 1), (6, 8), (14, 7), (15, 0)]:
        cols[..., bit] = y[..., j]
    yb_full = (
        np.packbits(cols.reshape(-1, 16), axis=-1, bitorder="little")
        .view(np.uint16)
        .reshape(B, S)
    )
    in_maps = []
    for i in range(NCORES):
        sl = slice(i * B_LOC, (i + 1) * B_LOC)
        D = np.empty((NPLANES, N), np.uint16)
        D[0] = yb_full[sl].reshape(-1)
        for j, pf in enumerate(planes_full):
            D[1 + j] = pf[sl].reshape(-1)
        in_maps.append({"D": D.reshape(-1).view(ml_dtypes.bfloat16)})
    return in_maps


def kernel(**inputs) -> np.ndarray:
    nc = _get_nc()
    in_maps = _shard_inputs(inputs)
    res = run_bass_kernel_spmd(nc, in_maps, list(range(NCORES)))
    total = 0.0
    for r in res.results:
        a = r["acc"].astype(np.float64)
        a1 = a[:, :K].sum()            # sum L_s
        a2a = a[:, K : 2 * K].sum()    # sum u*L_s
        a2b = a[:, 2 * K : 3 * K].sum()  # sum u*L_Pi
        a3 = a[:, 3 * K : 4 * K].sum()   # sum L_Pi
        # sum s*X = sum u*X + 0.5*sum X
        total += W0 * a1 + C2 * (a2a + 0.5 * a1) + (a2b + 0.5 * a3)
    mean = -total / float(B * S)
    return np.array([mean], dtype=np.float32)


# revision 34
# speedup vs baseline: 1.0761x; 1.0761x over previous
"""Trainium2 Bass kernel for nn_Loss_1_8323646620405 (multi-head BCE/CCE loss).

Data-parallel over batch: 8 cores x 8 batches. Host re-encodes inputs into a
single plane-major uint16 array per core (packbits labels + bf16 pred planes,
binary-head planes pre-shifted by -0.5) so each chunk is ONE DMA. Device:
  s    = any(y)        -> u = s-0.5            (DVE tensor_scalar on int16)
  L_s  = ln(0.5+2(ps-.5)u)                      (ACT Ln, accum -> acc1)
  ppe  = 0.5-2(pp-.5)(y0-.5), phe likewise      (DVE TT + ACT affine copy)
  Psel = P3[point idx], Qsel = Q4[serve idx]    (copy_predicated cascades, Pool)
  Pi   = ppe*phe*Psel*Qsel ; L_Pi = ln(Pi)      (DVE TT, ACT Ln)
  acc2 = sum s*(C2*L_s + L_Pi)                  (DVE STT accum)
loss = -(W0*acc1 + acc2)/(B*S) summed on host.
"""

import numpy as np
import ml_dtypes

import concourse.bass as bass
import concourse.mybir as mybir
import concourse.tile as tile
from concourse.bass_utils import run_bass_kernel_spmd

# ---- walrus single-wait workaround ----------------------------------------
# This container's walrus build encodes at most ONE semaphore wait per
# instruction ('Too many sync wait commands'). Tile's scheduler freely
# attaches N waits to one instruction. Two patches:
#  1. postorder_instruction_blocks wrapper: split any instruction carrying
#     >1 wait -- extra waits move to same-engine NoOps inserted before it.
#  2. _drain_and_barrier: one drain per outstanding logical processor.
import bass_rust
from concourse.tile_cfg import postorder_instruction_blocks as _orig_post

_DMA_PROC_START = 10  # Collectives/DMASW*/DMAHW* procs inc by 16 per tick
_nop_ctr = [0]


def _split_waits_in_list(insts):
    out = []
    for ins in insts:
        si = getattr(ins, "sync_info", None)
        waits = list(si.on_wait) if si is not None else []
        if len(waits) > 1:
            for w in waits[:-1]:
                _nop_ctr[0] += 1
                nop = mybir.InstNoOp(name=f"WSPL-{_nop_ctr[0]}", ins=[], outs=[])
                nop.engine = ins.engine
                nop.sync_info = bass_rust.SyncInfo(on_wait=[w], on_update=[])
                out.append(nop)
            ins.sync_info = bass_rust.SyncInfo(
                on_wait=[waits[-1]], on_update=list(si.on_update)
            )
        out.append(ins)
    return out


def _patched_post(instructions, start_bb, output):
    for k in list(instructions.keys()):
        instructions[k] = _split_waits_in_list(instructions[k])
    return _orig_post(instructions, start_bb, output)


def _split_drain_and_barrier(self, tick_clock, wait_clock):
    gc = tick_clock.global_clock
    alloc = wait_clock.sems.allocated()
    for proc in sorted(alloc):
        tick = gc.peek_next(proc) - 1
        if tick <= 0:
            continue
        scale = 16 if proc >= _DMA_PROC_START else 1
        d = self.nc.sync.drain()
        d.wait_op(alloc[proc], tick * scale, "sem-ge")

    self.nc.all_engine_barrier()
    popped = self.nc._tile_sem_poison_stack.pop()
    assert popped is self._sem_poison
    self.nc.clear_and_free_semaphores(list(self.sems.allocated().values()))
    self.nc.all_engine_barrier()


tile.postorder_instruction_blocks = _patched_post
tile.TileContext._drain_and_barrier = _split_drain_and_barrier

# ---- problem constants -----------------------------------------------------
B, S, F = 64, 32768, 9
W0, W1 = 0.51, 19.05
C2 = W1 - W0

NCORES = 8
B_LOC = B // NCORES          # 8 batches per core
N = B_LOC * S                # 262144 elements per core
P = 128                      # SBUF partitions
FD = N // P                  # 2048 free-dim elements per partition
NPLANES = 11                 # 0: labels, 1-10: bf16 pred planes
# >=256-col chunks keep DMA rows >=512B (full DMA rate)
CHUNKS = [(0, 256), (256, 640), (896, 640), (1536, 512)]
K = len(CHUNKS)

f32 = mybir.dt.float32
bf16 = mybir.dt.bfloat16
i16 = mybir.dt.int16
u16 = mybir.dt.uint16
Alu = mybir.AluOpType
Act = mybir.ActivationFunctionType


def _build_nc() -> bass.Bass:
    nc = bass.Bass()

    # const AP for Ln bias=0.5 (same pattern as Bass.__init__ consts)
    c05 = nc.alloc_sbuf_tensor("const-float32-0.5", [P, 1], f32)
    nc.gpsimd.memset(c05.ap(), 0.5)
    nc.const_aps.aps[(f32, 0.5)] = c05.ap()
    nc.all_engine_barrier()

    D_d = nc.declare_dram_parameter("D", [NPLANES * N], bf16, isOutput=False)
    acc_d = nc.declare_dram_parameter("acc", [P, 3 * K], f32, isOutput=True)

    Dv = D_d.rearrange("(k p c) -> p k c", k=NPLANES, p=P)

    with tile.TileContext(nc) as tc:
        with (
            tc.tile_pool(name="io", bufs=2) as io,
            tc.tile_pool(name="tmp", bufs=2) as tp,
            tc.tile_pool(name="acc", bufs=1) as ac,
        ):
            accT = ac.tile([P, 3 * K], f32)
            acc1 = accT[:, 0:K]             # sum L_s
            acc2 = accT[:, K : 2 * K]       # sum u*(C2*L_s + L_Pi)
            acc3 = accT[:, 2 * K : 3 * K]   # sum L_Pi

            tiles = {}

            def load(k):
                off, sz = CHUNKS[k]
                T = io.tile([P, NPLANES, sz], bf16, tag="T")
                if k == 0:
                    # labels first: the 9 label-only DVE ops can start while
                    # the 10 pred planes are still in flight
                    nc.sync.dma_start(T[:, 0:1, :], Dv[:, 0:1, off : off + sz])
                    nc.sync.dma_start(T[:, 1:, :], Dv[:, 1:, off : off + sz])
                else:
                    nc.sync.dma_start(T[:], Dv[:, :, off : off + sz])
                tiles[k] = T

            load(0)
            for k, (off, sz) in enumerate(CHUNKS):
                T = tiles.pop(k)
                if k + 1 < K:
                    load(k + 1)
                yb = T[:, 0, :].bitcast(u16)
                psh = T[:, 1, :]   # ps - 0.5 (host pre-shifted)
                pph = T[:, 2, :]   # pp - 0.5
                phh = T[:, 3, :]   # ph - 0.5
                P0 = T[:, 4, :]
                P1 = T[:, 5, :]
                Psel = T[:, 6, :]  # P2 plane, overwritten in place by selects
                Q0 = T[:, 7, :]
                Q1 = T[:, 8, :]
                Q2 = T[:, 9, :]
                Qsel = T[:, 10, :]  # Q3 plane, overwritten in place

                u = tp.tile([P, sz], bf16, tag="u")
                v0 = tp.tile([P, sz], bf16, tag="v0")
                u7m = tp.tile([P, sz], u16, tag="u7m")
                v7 = tp.tile([P, sz], bf16, tag="v7")
                y5m = tp.tile([P, sz], u16, tag="y5m")
                y4m = tp.tile([P, sz], u16, tag="y4m")
                y6m = tp.tile([P, sz], u16, tag="y6m")
                y3m = tp.tile([P, sz], u16, tag="y3m")
                y2m = tp.tile([P, sz], u16, tag="y2m")
                m_s = tp.tile([P, sz], bf16, tag="m_s")
                Xp = tp.tile([P, sz], bf16, tag="Xp")
                Xh = tp.tile([P, sz], bf16, tag="Xh")
                L_s = tp.tile([P, sz], bf16, tag="L_s")
                ppe = tp.tile([P, sz], bf16, tag="ppe")
                phe = tp.tile([P, sz], bf16, tag="phe")
                pr1 = tp.tile([P, sz], bf16, tag="pr1")
                pr2 = tp.tile([P, sz], bf16, tag="pr2")
                Pi = tp.tile([P, sz], bf16, tag="Pi")
                L_Pi = tp.tile([P, sz], bf16, tag="L_Pi")
                Ls2 = tp.tile([P, sz], bf16, tag="Ls2")
                R = tp.tile([P, sz], bf16, tag="R")
                sR = tp.tile([P, sz], bf16, tag="sR")
                dq1 = tp.tile([P, sz], bf16, tag="dq1")

                # label-derived masks. yb is uint16 with host bit layout:
                # bit15=y0, bit14=y7, bits0-4 = y2,y3,y4,y5,y6. Unsigned
                # compares extract the high bits in one tensor_scalar each
                # (walrus requires op0/op1 to share the bitwise/arith class).
                nc.vector.tensor_scalar(u[:], yb, 0, 0.5, Alu.is_gt, Alu.subtract)
                nc.vector.tensor_scalar(v0[:], yb, 32767, 0.5, Alu.is_gt, Alu.subtract)
                nc.vector.tensor_scalar(u7m[:], yb, 16384, None, Alu.bitwise_and)
                nc.vector.tensor_scalar(v7[:], u7m[:], 8192.0, None, Alu.subtract)
                nc.vector.tensor_scalar(y5m[:], yb, 8, None, Alu.bitwise_and)
                nc.vector.tensor_scalar(y4m[:], yb, 4, None, Alu.bitwise_and)
                nc.vector.tensor_scalar(y6m[:], yb, 16, None, Alu.bitwise_and)
                nc.vector.tensor_scalar(y3m[:], yb, 2, None, Alu.bitwise_and)
                nc.vector.tensor_scalar(y2m[:], yb, 1, None, Alu.bitwise_and)

                # stroke: L_s = ln(0.5 + 2*(ps-0.5)*u), accumulate per chunk
                nc.gpsimd.tensor_tensor(m_s[:], psh, u[:], op=Alu.mult)
                nc.scalar.activation(L_s[:], m_s[:], Act.Ln, bias=0.5, scale=2.0,
                                     accum_out=acc1[:, k : k + 1])
                # player: v0 = y0-0.5, Xp = (pp-.5)(y0-.5), ppe = 0.5-2*Xp
                nc.gpsimd.tensor_tensor(Xp[:], pph, v0[:], op=Alu.mult)
                nc.scalar.activation(ppe[:], Xp[:], Act.Copy, bias=0.5, scale=-2.0)
                # hand: v7 = 16384*(y7-0.5), phe = 0.5 - Xh/8192
                nc.gpsimd.tensor_tensor(Xh[:], phh, v7[:], op=Alu.mult)
                nc.scalar.activation(phe[:], Xh[:], Act.Copy, bias=0.5, scale=-1.0 / 8192.0)
                # point select: y4 ? P0 : (y5 ? P1 : P2)  (in place on P2 plane;
                # copy_predicated is DVE-only on Trn2)
                nc.vector.copy_predicated(Psel, y5m[:], P1)
                nc.vector.copy_predicated(Psel, y4m[:], P0)
                # serve select: y2 ? Q0 : (y3 ? Q1 : (y6 ? Q2 : Q3))
                nc.vector.copy_predicated(Qsel, y6m[:], Q2)
                nc.vector.copy_predicated(Qsel, y3m[:], Q1)
                nc.vector.copy_predicated(Qsel, y2m[:], Q0)
                # Pi = ppe*phe*Psel*Qsel ; L_Pi = ln(Pi)
                nc.gpsimd.tensor_tensor(pr1[:], ppe[:], phe[:], op=Alu.mult)
                nc.gpsimd.tensor_tensor(pr2[:], Psel, Qsel, op=Alu.mult)
                nc.gpsimd.tensor_tensor(Pi[:], pr1[:], pr2[:], op=Alu.mult)
                nc.scalar.activation(L_Pi[:], Pi[:], Act.Ln,
                                     accum_out=acc3[:, k : k + 1])
                # masked accum via u=s-0.5: sum s*R = sum u*R + sum R / 2,
                # host reassembles from acc1/acc3. R = C2*L_s + L_Pi.
                nc.vector.tensor_scalar(Ls2[:], L_s[:], C2, None, Alu.mult)
                nc.gpsimd.tensor_tensor(R[:], Ls2[:], L_Pi[:], op=Alu.add)
                nc.gpsimd.tensor_tensor(sR[:], u[:], R[:], op=Alu.mult)
                nc.scalar.activation(dq1[:], sR[:], Act.Copy,
                                     accum_out=acc2[:, k : k + 1])

            nc.sync.dma_start(acc_d[:], accT[:])

    return nc


_NC_CACHE = None


def _get_nc():
    global _NC_CACHE
    if _NC_CACHE is None:
        _NC_CACHE = _build_nc()
    return _NC_CACHE


def _shard_inputs(inputs):
    bf = ml_dtypes.bfloat16
    # bf16 planes (computed once on full arrays, then sliced per core)
    planes_full = [
        (inputs["y_pred_stroke"][..., 0] - 0.5).astype(bf).view(np.uint16),
        (inputs["y_pred_player"][..., 0] - 0.5).astype(bf).view(np.uint16),
        (inputs["y_pred_hand"][..., 0] - 0.5).astype(bf).view(np.uint16),
    ] + [
        inputs["y_pred_point"][..., j].astype(bf).view(np.uint16) for j in range(3)
    ] + [
        inputs["y_pred_serve"][..., j].astype(bf).view(np.uint16) for j in range(4)
    ]
    # labels -> uint16 with layout: bits0-4 = y2,y3,y4,y5,y6; bit5=y1;
    # bit6=y8; bit14=y7; bit15=y0 (high bits enable unsigned-compare tricks)
    y = inputs["y_target"].astype(np.uint8)
    cols = np.zeros(y.shape[:-1] + (16,), np.uint8)
    for bit, j in [(0, 2), (1, 3), (2, 4), (3, 5), (4, 6), (5, 1), (6, 8), (14, 7), (15, 0)]:
        cols[..., bit] = y[..., j]
    yb_full = (
        np.packbits(cols.reshape(-1, 16), axis=-1, bitorder="little")
        .view(np.uint16)
        .reshape(B, S)
    )
    in_maps = []
    for i in range(NCORES):
        sl = slice(i * B_LOC, (i + 1) * B_LOC)
        D = np.empty((NPLANES, N), np.uint16)
        D[0] = yb_full[sl].reshape(-1)
        for j, pf in enumerate(planes_full):
            D[1 + j] = pf[sl].reshape(-1)
        in_maps.append({"D": D.reshape(-1).view(ml_dtypes.bfloat16)})
    return in_maps


def kernel(**inputs) -> np.ndarray:
    nc = _get_nc()
    in_maps = _shard_inputs(inputs)
    res = run_bass_kernel_spmd(nc, in_maps, list(range(NCORES)))
    total = 0.0
    for r in res.results:
        a = r["acc"].astype(np.float64)
        a1 = a[:, :K].sum()              # sum L_s
        a2 = a[:, K : 2 * K].sum()       # sum u*(C2*L_s + L_Pi)
        a3 = a[:, 2 * K : 3 * K].sum()   # sum L_Pi
        # sum s*X = sum u*X + 0.5*sum X
        total += (W0 + 0.5 * C2) * a1 + 0.5 * a3 + a2
    mean = -total / float(B * S)
    return np.array([mean], dtype=np.float32)


# revision 59
# speedup vs baseline: 1.3515x; 1.2559x over previous
"""Trainium2 Bass kernel for nn_Loss_1_8323646620405 (multi-head BCE/CCE loss).

Data-parallel over batch: 8 cores x 8 batches. Host re-encodes inputs into a
single plane-major uint16 array per core (packbits labels + bf16 pred planes,
binary-head planes pre-shifted by -0.5) so each chunk is ONE DMA. Device:
  s    = any(y)        -> u = s-0.5            (DVE tensor_scalar on int16)
  L_s  = ln(0.5+2(ps-.5)u)                      (ACT Ln, accum -> acc1)
  ppe  = 0.5-2(pp-.5)(y0-.5), phe likewise      (DVE TT + ACT affine copy)
  Psel = P3[point idx], Qsel = Q4[serve idx]    (copy_predicated cascades, Pool)
  Pi   = ppe*phe*Psel*Qsel ; L_Pi = ln(Pi)      (DVE TT, ACT Ln)
  acc2 = sum s*(C2*L_s + L_Pi)                  (DVE STT accum)
loss = -(W0*acc1 + acc2)/(B*S) summed on host.
"""

import numpy as np
import ml_dtypes

import concourse.bass as bass
import concourse.mybir as mybir
import concourse.tile as tile
from concourse.bass_utils import run_bass_kernel_spmd

# ---- walrus single-wait workaround ----------------------------------------
# This container's walrus build encodes at most ONE semaphore wait per
# instruction ('Too many sync wait commands'). Tile's scheduler freely
# attaches N waits to one instruction. Two patches:
#  1. postorder_instruction_blocks wrapper: split any instruction carrying
#     >1 wait -- extra waits move to same-engine NoOps inserted before it.
#  2. _drain_and_barrier: one drain per outstanding logical processor.
import bass_rust
from concourse.tile_cfg import postorder_instruction_blocks as _orig_post

_DMA_PROC_START = 10  # Collectives/DMASW*/DMAHW* procs inc by 16 per tick
_nop_ctr = [0]


def _split_waits_in_list(insts):
    out = []
    for ins in insts:
        si = getattr(ins, "sync_info", None)
        waits = list(si.on_wait) if si is not None else []
        if len(waits) > 1:
            for w in waits[:-1]:
                _nop_ctr[0] += 1
                nop = mybir.InstNoOp(name=f"WSPL-{_nop_ctr[0]}", ins=[], outs=[])
                nop.engine = ins.engine
                nop.sync_info = bass_rust.SyncInfo(on_wait=[w], on_update=[])
                out.append(nop)
            ins.sync_info = bass_rust.SyncInfo(
                on_wait=[waits[-1]], on_update=list(si.on_update)
            )
        out.append(ins)
    return out


def _patched_post(instructions, start_bb, output):
    for k in list(instructions.keys()):
        instructions[k] = _split_waits_in_list(instructions[k])
    return _orig_post(instructions, start_bb, output)


def _split_drain_and_barrier(self, tick_clock, wait_clock):
    gc = tick_clock.global_clock
    alloc = wait_clock.sems.allocated()
    for proc in sorted(alloc):
        tick = gc.peek_next(proc) - 1
        if tick <= 0:
            continue
        scale = 16 if proc >= _DMA_PROC_START else 1
        d = self.nc.sync.drain()
        d.wait_op(alloc[proc], tick * scale, "sem-ge")

    self.nc.all_engine_barrier()
    popped = self.nc._tile_sem_poison_stack.pop()
    assert popped is self._sem_poison
    self.nc.clear_and_free_semaphores(list(self.sems.allocated().values()))
    self.nc.all_engine_barrier()


tile.postorder_instruction_blocks = _patched_post
tile.TileContext._drain_and_barrier = _split_drain_and_barrier

# ---- problem constants -----------------------------------------------------
B, S, F = 64, 32768, 9
W0, W1 = 0.51, 19.05
C2 = W1 - W0

NCORES = 8
B_LOC = B // NCORES          # 8 batches per core
N = B_LOC * S                # 262144 elements per core
P = 128                      # SBUF partitions
FD = N // P                  # 2048 free-dim elements per partition
NPLANES = 11                 # 0: labels, 1-10: bf16 pred planes
# >=256-col chunks keep DMA rows >=512B (full DMA rate)
CHUNKS = [(0, 512), (512, 512), (1024, 512), (1536, 512)]
K = len(CHUNKS)

f32 = mybir.dt.float32
bf16 = mybir.dt.bfloat16
i16 = mybir.dt.int16
u16 = mybir.dt.uint16
u8 = mybir.dt.uint8
f8 = mybir.dt.float8e4
Alu = mybir.AluOpType
Act = mybir.ActivationFunctionType


def _build_nc() -> bass.Bass:
    nc = bass.Bass()

    # const AP for Ln bias=0.5 (same pattern as Bass.__init__ consts)
    c05 = nc.alloc_sbuf_tensor("const-float32-0.5", [P, 1], f32)
    nc.gpsimd.memset(c05.ap(), 0.5)
    nc.const_aps.aps[(f32, 0.5)] = c05.ap()
    nc.all_engine_barrier()

    # D16: 2-byte planes [labels, ps-.5, pp-.5, ph-.5]; D8: 1-byte planes
    # [m2, m3, m4, m5, m6, P0, P1, P2, Q0, Q1, Q2, Q3] (masks u8, probs fp8)
    D16_d = nc.declare_dram_parameter("D16", [4 * N], bf16, isOutput=False)
    D8_d = nc.declare_dram_parameter("D8", [12 * N], u8, isOutput=False)
    acc_d = nc.declare_dram_parameter("acc", [P, 3 * K], f32, isOutput=True)

    V16 = D16_d.rearrange("(k p c) -> p k c", k=4, p=P)
    V8 = D8_d.rearrange("(k p c) -> p k c", k=12, p=P)

    with tile.TileContext(nc) as tc:
        with (
            tc.tile_pool(name="io", bufs=3) as io,
            tc.tile_pool(name="tmp", bufs=2) as tp,
            tc.tile_pool(name="acc", bufs=1) as ac,
        ):
            accT = ac.tile([P, 3 * K], f32)
            acc1 = accT[:, 0:K]             # sum L_s
            acc2 = accT[:, K : 2 * K]       # sum u*(C2*L_s + L_Pi)
            acc3 = accT[:, 2 * K : 3 * K]   # sum L_Pi

            tiles = {}

            def load(k):
                off, sz = CHUNKS[k]
                T16 = io.tile([P, 4, sz], bf16, tag="T16")
                T8 = io.tile([P, 12, sz], u8, tag="T8")
                nc.sync.dma_start(T16[:], V16[:, :, off : off + sz])
                nc.sync.dma_start(T8[:], V8[:, :, off : off + sz])
                tiles[k] = (T16, T8)

            load(0)
            for k, (off, sz) in enumerate(CHUNKS):
                T16, T8 = tiles.pop(k)
                if k + 1 < K:
                    load(k + 1)
                yb = T16[:, 0, :].bitcast(u16)
                psh = T16[:, 1, :]  # ps - 0.5 (host pre-shifted)
                pph = T16[:, 2, :]  # pp - 0.5
                phh = T16[:, 3, :]  # ph - 0.5
                y2m = T8[:, 0, :]
                y3m = T8[:, 1, :]
                y4m = T8[:, 2, :]
                y5m = T8[:, 3, :]
                y6m = T8[:, 4, :]
                P0 = T8[:, 5, :].bitcast(f8)
                P1 = T8[:, 6, :].bitcast(f8)
                Psel = T8[:, 7, :].bitcast(f8)  # P2, overwritten in place
                Q0 = T8[:, 8, :].bitcast(f8)
                Q1 = T8[:, 9, :].bitcast(f8)
                Q2 = T8[:, 10, :].bitcast(f8)
                Qsel = T8[:, 11, :].bitcast(f8)  # Q3, overwritten in place

                u = tp.tile([P, sz], bf16, tag="u")
                v0 = tp.tile([P, sz], bf16, tag="v0")
                u7m = tp.tile([P, sz], u16, tag="u7m")
                v7 = tp.tile([P, sz], bf16, tag="v7")
                m_s = tp.tile([P, sz], bf16, tag="m_s")
                Xp = tp.tile([P, sz], bf16, tag="Xp")
                Xh = tp.tile([P, sz], bf16, tag="Xh")
                L_s = tp.tile([P, sz], bf16, tag="L_s")
                ppe = tp.tile([P, sz], bf16, tag="ppe")
                phe = tp.tile([P, sz], bf16, tag="phe")
                pr1 = tp.tile([P, sz], bf16, tag="pr1")
                pr2 = tp.tile([P, sz], bf16, tag="pr2")
                Pi = tp.tile([P, sz], bf16, tag="Pi")
                L_Pi = tp.tile([P, sz], bf16, tag="L_Pi")
                Ls2 = tp.tile([P, sz], bf16, tag="Ls2")
                R = tp.tile([P, sz], bf16, tag="R")
                sR = tp.tile([P, sz], bf16, tag="sR")
                dq1 = tp.tile([P, sz], bf16, tag="dq1")

                # label-derived masks. yb is uint16 with host bit layout:
                # bit15=y0, bit14=y7, bits0-4 = y2,y3,y4,y5,y6. Unsigned
                # compares extract the high bits in one tensor_scalar each
                # (walrus requires op0/op1 to share the bitwise/arith class).
                last = k == K - 1
                # label ops first (T16 lands before T8), then selects
                nc.vector.tensor_scalar(u[:], yb, 0, 0.5, Alu.is_gt, Alu.subtract)
                nc.vector.tensor_scalar(v0[:], yb, 32767, 0.5, Alu.is_gt, Alu.subtract)
                nc.vector.tensor_scalar(u7m[:], yb, 16384, None, Alu.bitwise_and)
                nc.vector.tensor_scalar(v7[:], u7m[:], 8192.0, None, Alu.subtract)
                # point select: y4 ? P0 : (y5 ? P1 : P2)  (in place on P2 plane;
                # copy_predicated is DVE-only on Trn2)
                nc.vector.copy_predicated(Psel, y5m, P1)
                nc.vector.copy_predicated(Psel, y4m, P0)
                # serve select: y2 ? Q0 : (y3 ? Q1 : (y6 ? Q2 : Q3))
                nc.vector.copy_predicated(Qsel, y6m, Q2)
                nc.vector.copy_predicated(Qsel, y3m, Q1)
                nc.vector.copy_predicated(Qsel, y2m, Q0)

                # stroke: L_s = ln(0.5 + 2*(ps-0.5)*u), accumulate per chunk
                nc.gpsimd.tensor_tensor(m_s[:], psh, u[:], op=Alu.mult)
                nc.scalar.activation(L_s[:], m_s[:], Act.Ln, bias=0.5, scale=2.0,
                                     accum_out=acc1[:, k : k + 1])
                # player: v0 = y0-0.5, Xp = (pp-.5)(y0-.5), ppe = 0.5-2*Xp
                nc.gpsimd.tensor_tensor(Xp[:], pph, v0[:], op=Alu.mult)
                nc.scalar.activation(ppe[:], Xp[:], Act.Copy, bias=0.5, scale=-2.0)
                # hand: v7 = 16384*(y7-0.5), phe = 0.5 - Xh/8192
                nc.gpsimd.tensor_tensor(Xh[:], phh, v7[:], op=Alu.mult)
                nc.scalar.activation(phe[:], Xh[:], Act.Copy, bias=0.5, scale=-1.0 / 8192.0)
                # Pi = ppe*phe*Psel*Qsel ; L_Pi = ln(Pi)
                tt2 = nc.vector if last else nc.gpsimd
                nc.gpsimd.tensor_tensor(pr1[:], ppe[:], phe[:], op=Alu.mult)
                tt2.tensor_tensor(pr2[:], Psel, Qsel, op=Alu.mult)
                tt2.tensor_tensor(Pi[:], pr1[:], pr2[:], op=Alu.mult)
                nc.scalar.activation(L_Pi[:], Pi[:], Act.Ln,
                                     accum_out=acc3[:, k : k + 1])
                # masked accum via u=s-0.5: sum s*R = sum u*R + sum R / 2,
                # host reassembles from acc1/acc3. R = C2*L_s + L_Pi.
                # Last chunk: keep the whole drain chain on DVE (cross-engine
                # hops would serialize the tail).
                nc.vector.tensor_scalar(Ls2[:], L_s[:], C2, None, Alu.mult)
                if last:
                    nc.vector.tensor_tensor(R[:], Ls2[:], L_Pi[:], op=Alu.add)
                    nc.vector.tensor_tensor(sR[:], u[:], R[:], op=Alu.mult)
                    nc.vector.tensor_scalar(
                        dq1[:], sR[:], 0.0, None, Alu.add, Alu.add,
                        accum_out=acc2[:, k : k + 1],
                    )
                else:
                    nc.gpsimd.tensor_tensor(R[:], Ls2[:], L_Pi[:], op=Alu.add)
                    nc.gpsimd.tensor_tensor(sR[:], u[:], R[:], op=Alu.mult)
                    nc.scalar.activation(dq1[:], sR[:], Act.Copy,
                                         accum_out=acc2[:, k : k + 1])

            nc.sync.dma_start(acc_d[:], accT[:])

    return nc


_NC_CACHE = None


def _get_nc():
    global _NC_CACHE
    if _NC_CACHE is None:
        _NC_CACHE = _build_nc()
    return _NC_CACHE


def _shard_inputs(inputs):
    bf = ml_dtypes.bfloat16
    e4m3 = ml_dtypes.float8_e4m3fn
    # binary-head planes, pre-shifted by -0.5, bf16
    bp_full = [
        (inputs["y_pred_stroke"][..., 0] - 0.5).astype(bf),
        (inputs["y_pred_player"][..., 0] - 0.5).astype(bf),
        (inputs["y_pred_hand"][..., 0] - 0.5).astype(bf),
    ]
    # select-probability planes, fp8 e4m3 (feed the big product's ln only)
    fp_full = [inputs["y_pred_point"][..., j].astype(e4m3) for j in range(3)] + [
        inputs["y_pred_serve"][..., j].astype(e4m3) for j in range(4)
    ]
    y = inputs["y_target"].astype(np.uint8)
    # select masks as u8 planes: y2, y3, y4, y5, y6
    m_full = [y[..., j] for j in (2, 3, 4, 5, 6)]
    # labels -> uint16: bit15=y0, bit14=y7, low bits = remaining labels so
    # (yb != 0) still means any(y)
    cols = np.zeros(y.shape[:-1] + (16,), np.uint8)
    for bit, j in [(0, 2), (1, 3), (2, 4), (3, 5), (4, 6), (5, 1), (6, 8), (14, 7), (15, 0)]:
        cols[..., bit] = y[..., j]
    yb_full = (
        np.packbits(cols.reshape(-1, 16), axis=-1, bitorder="little")
        .view(np.uint16)
        .reshape(B, S)
    )
    in_maps = []
    for i in range(NCORES):
        sl = slice(i * B_LOC, (i + 1) * B_LOC)
        D16 = np.empty((4, N), np.uint16)
        D16[0] = yb_full[sl].reshape(-1)
        for j, pf in enumerate(bp_full):
            D16[1 + j] = pf[sl].reshape(-1).view(np.uint16)
        D8 = np.empty((12, N), np.uint8)
        for j, pf in enumerate(m_full):
            D8[j] = pf[sl].reshape(-1)
        for j, pf in enumerate(fp_full):
            D8[5 + j] = pf[sl].reshape(-1).view(np.uint8)
        in_maps.append(
            {
                "D16": D16.reshape(-1).view(bf),
                "D8": D8.reshape(-1),
            }
        )
    return in_maps


def kernel(**inputs) -> np.ndarray:
    nc = _get_nc()
    in_maps = _shard_inputs(inputs)
    res = run_bass_kernel_spmd(nc, in_maps, list(range(NCORES)))
    total = 0.0
    for r in res.results:
        a = r["acc"].astype(np.float64)
        a1 = a[:, :K].sum()              # sum L_s
        a2 = a[:, K : 2 * K].sum()       # sum u*(C2*L_s + L_Pi)
        a3 = a[:, 2 * K : 3 * K].sum()   # sum L_Pi
        # sum s*X = sum u*X + 0.5*sum X
        total += (W0 + 0.5 * C2) * a1 + 0.5 * a3 + a2
    mean = -total / float(B * S)
    return np.array([mean], dtype=np.float32)
